# revision 6
# baseline (speedup 1.0000x reference)
"""CRF mean-log-likelihood kernel for Trainium2, 8 NeuronCores, data-parallel.

Problem: B=256, M=1024, D=128, N=26.
  e = X @ W.T ; prob-space forward scan f <- exp(e_i) * (exp(T)^T f);
  result = mean_j [ sum_p e[p, y_p] + sum_p T[y_p, y_p+1] - logZ_j ].

Sharding: batch across 8 cores (32 seqs/core). Device computes, per core:
  - e via PE (X^T-block stationary, W^T moving) in natural [pos, label] layout
  - emission-gather sums via GPSIMD indirect_copy + ACT Ln + DVE masked reduce
  - logZ via a chunked rank-1 two-pass scan (K=8 positions/chunk):
      pass1: w_c = P_c @ 1 for all chunks in parallel (8 serial steps)
      pass2: v_c = P_c @ w_{c-1}; logZ telescopes into log-ratios of column sums
    4 chunk-bands folded onto partitions (blockdiag exp(T) lhsT), so each scan
    step is one [128x128]x[128,cols] matmul + one DVE multiply by exp(e).
Host finishes with tiny assembly: logs of per-chunk sums, transition term,
and the mean. Heavy data (X) is touched only on device.

Position coords per seq: p = 512*Wseq + 128*fb + 8*sub + k
  (global W-block w = 2*j_core + Wseq; band fb = partition band of the chunk;
   chunk id c = 64*Wseq + 16*fb + sub; within-chunk step k).
"""
import sys
sys.path.insert(0, '/opt/trn_rl_repo')
import numpy as np
import ml_dtypes

bf16 = ml_dtypes.bfloat16

B, M, D, N = 256, 1024, 128, 26
NCORES = 8
S = B // NCORES          # 32 seqs per core
NWc = 2 * S              # 64 W-blocks (512 positions) per core
NGRP = 4                 # seq groups per core (pipelining granularity)
SG = S // NGRP           # 8 seqs per group
NWg = 2 * SG             # 16 W-blocks per group
K = 8                    # chunk length
NSUB = 16                # sub-chunks per (W, band)
COLS_G = NWg * NSUB      # 256 state columns per group
GATH_G = 16 * NWg * 4    # 1024 gather slots per group

_cache = {}


def _host_consts(W, T):
    Wt = np.ascontiguousarray(W.T).astype(bf16)              # [128, 26]
    Mt = np.exp(T.astype(np.float64))
    M4 = np.zeros((128, 128), np.float32)
    for g in range(4):
        M4[32 * g:32 * g + N, 32 * g:32 * g + N] = Mt
    P_up = np.zeros((128, 128), np.float32)
    for g in range(3):
        P_up[32 * g:32 * g + N, 32 * (g + 1):32 * (g + 1) + N] = np.eye(N)
    P_wrap = np.zeros((128, 128), np.float32)
    P_wrap[96:96 + N, 0:N] = np.eye(N)
    ones4 = np.zeros((128, 4), np.float32)
    for g in range(4):
        ones4[32 * g:32 * g + N, g] = 1.0
    rcs = (1.0 / Mt.sum(axis=0)).astype(np.float32)          # [26]
    rcs_tiled = np.tile(rcs, SG)[None, :]                    # [1, 8*26]
    mask = np.zeros((128, GATH_G), np.float32)
    for p in range(128):
        for i in range(NWg * 4):
            mask[p, 16 * i + (p % 16)] = 1.0
    return dict(Wt=Wt, M4=M4.astype(bf16), P_up=P_up.astype(bf16),
                P_wrap=P_wrap.astype(bf16), ones4=ones4.astype(bf16),
                rcs_tiled=rcs_tiled.astype(np.float32), mask=mask.astype(bf16))


def _gather_idx(labels_core):
    """uint16 [128, 256]: wrapped per-16-partition-group sequences.
    Slot s = 16*i + r of group gg picks, for partition p = 16*gg + r, block
    i = (w_rel, fb) of its seq-group, the column w_rel*128 + fb*32 + y[P]."""
    y = labels_core.reshape(-1)
    idx = np.zeros((128, 4 * NWc), np.uint16)     # [128, 256] (64 cols/group)
    for g in range(NGRP):
        for wrel in range(NWg):
            w = 16 * g + wrel
            for fb in range(4):
                i = wrel * 4 + fb
                P0 = w * 512 + fb * 128
                for r in range(16):
                    for gg in range(8):
                        p = 16 * gg + r
                        idx[p, 64 * g + i] = wrel * 128 + fb * 32 + y[P0 + p]
    return idx


def _build_kernel():
    import concourse.bass as bass
    import concourse.mybir as mybir
    import concourse.tile as tile
    from concourse.masks import make_identity

    fp32 = mybir.dt.float32
    bfl = mybir.dt.bfloat16

    nc = bass.Bass()
    X_in = nc.dram_tensor("X", [S * M, D], fp32, kind="ExternalInput")
    idx_in = nc.dram_tensor("gidx", [128, 4 * NWc], mybir.dt.uint16, kind="ExternalInput")
    Wt_in = nc.dram_tensor("Wt", [128, N], bfl, kind="ExternalInput")
    M4_in = nc.dram_tensor("M4", [128, 128], bfl, kind="ExternalInput")
    Pup_in = nc.dram_tensor("P_up", [128, 128], bfl, kind="ExternalInput")
    Pwr_in = nc.dram_tensor("P_wrap", [128, 128], bfl, kind="ExternalInput")
    ones4_in = nc.dram_tensor("ones4", [128, 4], bfl, kind="ExternalInput")
    rcs_in = nc.dram_tensor("rcs_tiled", [1, SG * N], fp32, kind="ExternalInput")
    mask_in = nc.dram_tensor("mask", [128, GATH_G], bfl, kind="ExternalInput")

    sw_out = nc.dram_tensor("s_w", [4, NWc * NSUB], fp32, kind="ExternalOutput")
    sv_out = nc.dram_tensor("s_v", [4, NWc * NSUB], fp32, kind="ExternalOutput")
    gs_out = nc.dram_tensor("gsum", [128, NGRP], fp32, kind="ExternalOutput")

    with tile.TileContext(nc) as tc:
        with tc.tile_pool(name="const", bufs=1) as cpool, \
             tc.tile_pool(name="xn", bufs=2) as xn_pool, \
             tc.tile_pool(name="xt", bufs=2) as xt_pool, \
             tc.tile_pool(name="un", bufs=2) as un_pool, \
             tc.tile_pool(name="u4p", bufs=1) as u4_pool, \
             tc.tile_pool(name="scr", bufs=2) as scr_pool, \
             tc.tile_pool(name="st", bufs=2) as st_pool, \
             tc.tile_pool(name="acc", bufs=1) as acc_pool, \
             tc.tile_pool(name="eps", bufs=2, space="PSUM") as eps_pool, \
             tc.tile_pool(name="tps", bufs=1, space="PSUM") as tps_pool, \
             tc.tile_pool(name="sps", bufs=2, space="PSUM") as sps_pool, \
             tc.tile_pool(name="mps", bufs=1, space="PSUM") as mps_pool:

            Wt_sb = cpool.tile([128, N], bfl)
            nc.sync.dma_start(Wt_sb[:], Wt_in[:])
            M4_sb = cpool.tile([128, 128], bfl)
            nc.sync.dma_start(M4_sb[:], M4_in[:])
            Pup_sb = cpool.tile([128, 128], bfl)
            nc.sync.dma_start(Pup_sb[:], Pup_in[:])
            Pwr_sb = cpool.tile([128, 128], bfl)
            nc.sync.dma_start(Pwr_sb[:], Pwr_in[:])
            ones4_sb = cpool.tile([128, 4], bfl)
            nc.sync.dma_start(ones4_sb[:], ones4_in[:])
            rcs_sb = cpool.tile([1, SG * N], fp32)
            nc.sync.dma_start(rcs_sb[:], rcs_in[:])
            mask_sb = cpool.tile([128, GATH_G], bfl)
            nc.sync.dma_start(mask_sb[:], mask_in[:])
            idx_sb = cpool.tile([128, 4 * NWc], mybir.dt.uint16)
            nc.sync.dma_start(idx_sb[:], idx_in[:])
            ident = cpool.tile([128, 128], bfl)
            make_identity(nc, ident[:])

            # persistent per-core buffers
            u4 = acc_pool.tile([128, NWc * 128], bfl)        # [32fb+a, w*128+pp]
            s_w = acc_pool.tile([4, NWc * NSUB], fp32)
            s_v = acc_pool.tile([4, NWc * NSUB], fp32)
            gsum = acc_pool.tile([128, NGRP], fp32)

            X_v = X_in[:].rearrange("(b p) d -> b p d", p=128)  # [256, 128, 128]

            for g in range(NGRP):
                # ---- load + cast X for this seq group: [128, 64*128] bf16
                xn = xn_pool.tile([128, NWg * 4, 128], bfl)
                nc.gpsimd.dma_start(
                    xn[:], X_v[g * 64:(g + 1) * 64].rearrange("b p d -> p b d"))
                # ---- transpose all 64 blocks: xt[dd, blk, pp] = xn[pp, blk, dd]
                xt = xt_pool.tile([128, NWg * 4, 128], bfl)
                nc.sync.dma_start_transpose(
                    xt[:], xn[:].rearrange("p b d -> p (b d)"))

                # ---- e-matmul + exp-copy into u_nat
                un = un_pool.tile([128, NWg * 128], bfl)     # [pp, wrel*128+fb*32+a]
                nc.gpsimd.memset(un[:].rearrange("p (w f a) -> p w f a", f=4, a=32)
                                 [:, :, :, 26:32], 0.0)      # zero label-pad cols
                esb = un_pool.tile([128, NWg * 128], bfl)    # raw e, same layout
                for q in range(4):                           # 16 blocks per psum tile
                    ep = eps_pool.tile([128, 16 * N], fp32)
                    for bi in range(16):
                        blk = q * 16 + bi
                        nc.tensor.matmul(ep[:, bi * N:(bi + 1) * N],
                                         xt[:, blk, :], Wt_sb[:],
                                         start=True, stop=True)
                    # exp-copy: out cols (wrel_q*128 + fb*32 + a) 3-level AP
                    dst = un[:].rearrange("p (w f a) -> p w f a", f=4, a=32) \
                        [:, q * 4:(q + 1) * 4, :, 0:N]
                    src_ap = ep[:].rearrange(
                        "p (w f a) -> p w f a", f=4, a=N)[:, 0:4, :, :]
                    nc.scalar.activation(dst, src_ap,
                                         mybir.ActivationFunctionType.Exp)
                    dst_e = esb[:].rearrange("p (w f a) -> p w f a", f=4, a=32) \
                        [:, q * 4:(q + 1) * 4, :, 0:N]
                    nc.scalar.copy(dst_e, src_ap)

                # ---- emission gather (before u0 fix)
                picked = scr_pool.tile([128, GATH_G], bfl)
                nc.gpsimd.indirect_copy(
                    picked[:], esb[:], idx_sb[:, g * 64:(g + 1) * 64],
                    i_know_ap_gather_is_preferred=True)
                masked = scr_pool.tile([128, GATH_G], fp32)
                nc.vector.tensor_mul(masked[:], picked[:], mask_sb[:])
                nc.vector.tensor_reduce(gsum[:, g:g + 1], masked[:],
                                        mybir.AxisListType.X,
                                        mybir.AluOpType.add)

                # ---- u0 fix: position 0 of each seq in group
                nc.vector.tensor_mul(
                    un[0:1].rearrange("p (j c) -> p j c", c=256)[:, :, 0:N],
                    un[0:1].rearrange("p (j c) -> p j c", c=256)[:, :, 0:N],
                    rcs_sb[:].rearrange("p (j a) -> p j a", a=N))

                # ---- band transposes into u4
                for t in range(4):                           # 4 W-blocks per psum tile
                    tp = tps_pool.tile([128, 4, 128], bfl)
                    for wi in range(4):
                        wrel = t * 4 + wi
                        nc.tensor.transpose(tp[:, wi, :],
                                            un[:, wrel * 128:(wrel + 1) * 128],
                                            ident[:])
                    w0 = 16 * g + t * 4
                    nc.vector.tensor_copy(
                        u4[:, w0 * 128:(w0 + 4) * 128], tp[:])

                # ---- two-pass chunked scan for this group
                u4g = u4[:].rearrange("q (w s k) -> q w s k", s=NSUB, k=K)

                def scan_pass(st_init):
                    st = st_init
                    for k in range(K):
                        mp = sps_pool.tile([128, COLS_G], fp32)
                        nc.tensor.matmul(mp[:], M4_sb[:], st[:],
                                         start=True, stop=True)
                        stn = st_pool.tile([128, COLS_G], bfl)
                        uop = u4g[:, 16 * g:16 * g + NWg, :, k] \
                            .rearrange("q w s -> q (w s)")
                        nc.vector.tensor_mul(stn[:], mp[:], uop)
                        st = stn
                    return st

                st1 = st_pool.tile([128, COLS_G], bfl)
                nc.gpsimd.memset(st1[:], 0.0)
                for fb in range(4):
                    nc.gpsimd.memset(st1[32 * fb:32 * fb + N, :], 1.0)
                wfin = scan_pass(st1)

                op = mps_pool.tile([4, COLS_G], fp32, tag="ones_out")
                nc.tensor.matmul(op[:], ones4_sb[:], wfin[:], start=True, stop=True)
                nc.vector.tensor_copy(s_w[:, g * COLS_G:(g + 1) * COLS_G], op[:])

                # pass-2 init: shift state by one chunk
                st2 = st_pool.tile([128, COLS_G], bfl)
                wv = wfin[:].rearrange("q (w s) -> q w s", s=NSUB)
                sv2 = st2[:].rearrange("q (w s) -> q w s", s=NSUB)
                nc.vector.tensor_copy(sv2[:, :, 1:NSUB], wv[:, :, 0:NSUB - 1])
                rp = mps_pool.tile([128, NWg], fp32, tag="rot_out")
                wv2 = wfin[:].rearrange("q (w2 e s) -> q w2 e s", e=2, s=NSUB)
                rp2 = rp[:].rearrange("q (w2 e) -> q w2 e", e=2)
                nc.tensor.matmul(rp[:], Pup_sb[:], wv[:, :, NSUB - 1],
                                 start=True, stop=False)
                nc.tensor.matmul(rp2[:, :, 1], Pwr_sb[:],
                                 wv2[:, :, 0, NSUB - 1],
                                 start=False, stop=True, skip_group_check=True)
                nc.vector.tensor_copy(sv2[:, :, 0], rp[:])
                sv2r = st2[:].rearrange("q (w2 e s) -> q w2 e s", e=2, s=NSUB)
                nc.gpsimd.memset(sv2r[0:N, :, 0, 0], 1.0)    # dummy init, c=0 chunks
                vfin = scan_pass(st2)

                ov = mps_pool.tile([4, COLS_G], fp32, tag="ones_out")
                nc.tensor.matmul(ov[:], ones4_sb[:], vfin[:], start=True, stop=True)
                nc.vector.tensor_copy(s_v[:, g * COLS_G:(g + 1) * COLS_G], ov[:])

            nc.sync.dma_start(sw_out[:], s_w[:])
            nc.sync.dma_start(sv_out[:], s_v[:])
            nc.sync.dma_start(gs_out[:], gsum[:])

    _split_multi_waits(nc)
    return nc


def _split_multi_waits(nc, max_waits=1):
    """This walrus build allows one sem-wait per instruction; split extras
    into standalone EventSemaphore instructions on the same engine."""
    import concourse.mybir as mybir
    import bass_rust
    total = 0
    for fn in nc.m.functions:
        for bb in fn.blocks:
            insts = list(bb.instructions)
            out = []
            for ins in insts:
                si = ins.sync_info
                if si is not None and len(si.on_wait) > max_waits:
                    waits = list(si.on_wait)
                    for j, w in enumerate(waits[max_waits:]):
                        ev = mybir.InstEventSemaphore(
                            name=f'{ins.name}-xw{j}', ins=[], outs=[])
                        ev.sync_info = bass_rust.SyncInfo(on_wait=[w], on_update=[])
                        ev.engine = ins.engine
                        out.append(ev)
                        total += 1
                    ins.sync_info = bass_rust.SyncInfo(
                        on_wait=waits[:max_waits], on_update=list(si.on_update))
                out.append(ins)
            bb.instructions.clear()
            bb.instructions.extend(out)
    return total


def kernel(X, labels, W, T, _trace=False):
    from concourse.bass_utils import run_bass_kernel_spmd

    if 'nc' not in _cache:
        _cache['nc'] = _build_kernel()
    nc = _cache['nc']

    consts = _host_consts(W, T)
    in_maps = []
    for c in range(NCORES):
        Xc = np.ascontiguousarray(X[S * c:S * (c + 1)]).reshape(S * M, D)
        m = {"X": Xc, "gidx": _gather_idx(labels[S * c:S * (c + 1)])}
        m.update(consts)
        in_maps.append(m)

    out = run_bass_kernel_spmd(nc, in_maps, core_ids=list(range(NCORES)),
                               trace=_trace)
    results = out.results

    total = 0.0
    for c in range(NCORES):
        r = results[c]
        s_w = np.asarray(r["s_w"], np.float64)       # [4, 1024], col = w*16+sub
        s_v = np.asarray(r["s_v"], np.float64)
        gsum = float(np.asarray(r["gsum"], np.float64).sum())
        ls_w, ls_v = np.log(s_w), np.log(s_v)
        total_v, total_w = ls_v.sum(), ls_w.sum()
        j = np.arange(S)
        corr_v = ls_v[0, 32 * j].sum()               # c=0 dummy cols (band0, w=2j, sub0)
        corr_w0 = ls_w[0, 32 * j].sum()
        corr_w127 = ls_w[3, (2 * j + 1) * 16 + 15].sum()   # c=127 (band3, w odd, sub15)
        logZ_sum = corr_w0 + (total_v - corr_v) - (total_w - corr_w127)
        yc = labels[S * c:S * (c + 1)]
        trans = float(T[yc[:, :-1], yc[:, 1:]].sum(dtype=np.float64))
        total += gsum + trans - logZ_sum

    res = np.float32(total / B)
    if _trace:
        return res, out
    return res
